# revision 15
# baseline (speedup 1.0000x reference)
"""Trainium2 Bass kernel for nn_Attention2d (sparse_attention).

Math (per reference):
  x: (2, 128, 64, 64); T = 4096 tokens; 4 heads x 32 channels.
  qkv 1x1-conv -> per-head attention over T -> 1x1-conv out proj -> residual.

Sharding: one (batch, head) pair per core (8 cores). Each core computes its
head's attention fully on-chip (flash-style streaming; no max-subtraction --
scores are O(5) so exp in fp32 is exact-safe) and returns the per-head
partial of the output projection; the host sums the 4 head partials per
batch and adds the residual + biases (exact).

Per-core structure (T=4096, t-blocks of 512, s-supers of 4x128):
  - q,k replicated x4 across partition groups so the K=32 score matmuls
    row-pack 4-wide into the PE array (tile_position) -> one (128s, 512t)
    score block per bank, 4 banks per super.
  - ScalarE exp's 2048 PSUM columns per instruction -> bf16 P in SBUF.
    This is the bottleneck: 16.7M exps/core at 1 elem/cycle/lane.
  - v is produced directly transposed (x_block stationary) with a 32-wide
    ones block appended, so each PV matmul also emits the softmax
    denominator replicated over partitions 32..63.
  - PV accumulates into bank 0 of the same (by then exp-consumed) score
    tile; VectorE flushes per super into an SBUF accumulator.
  - Emission is software-pipelined: S_T(j+1) is emitted before PV(j) so the
    Tensor engine FIFO never blocks the next exp behind PV's wait.
"""

import numpy as np
import ml_dtypes

B, C, Hh, Ww = 2, 128, 64, 64
T = Hh * Ww          # 4096
NH, CH = 4, 32
SCALE2 = float(1.0 / np.sqrt(CH))
N_CORES = 8
NSUP = T // 512      # 8 supers per t-block, 8 t-blocks

_cache = {}


def _build_nc(debug=False):
    import concourse.tile as tile
    from concourse import bacc, mybir

    BF16 = mybir.dt.bfloat16
    F32 = mybir.dt.float32
    Exp = mybir.ActivationFunctionType.Exp

    nc = bacc.Bacc("TRN2", target_bir_lowering=False, debug=False,
                   num_devices=N_CORES)
    dbg = {}
    if debug:
        dbg["q"] = nc.dram_tensor("dq", [128, T], BF16, kind="ExternalOutput")
        dbg["k"] = nc.dram_tensor("dk", [128, T], BF16, kind="ExternalOutput")
        dbg["vT"] = nc.dram_tensor("dvT", [128, 2048], BF16, kind="ExternalOutput")
        dbg["an"] = nc.dram_tensor("dan", [32, T], BF16, kind="ExternalOutput")
        dbg["acc"] = nc.dram_tensor("dacc", [64, 512], F32, kind="ExternalOutput")
        dbg["p"] = nc.dram_tensor("dp", [128, 2048], BF16, kind="ExternalOutput")

    x_in = nc.dram_tensor("x", [128, T], BF16, kind="ExternalInput")
    wq_in = nc.dram_tensor("wqT", [128, 128], BF16, kind="ExternalInput")
    wk_in = nc.dram_tensor("wkT", [128, 128], BF16, kind="ExternalInput")
    wv_in = nc.dram_tensor("wvT", [128, 32], BF16, kind="ExternalInput")
    wp_in = nc.dram_tensor("wpT", [32, 128], BF16, kind="ExternalInput")
    bq_in = nc.dram_tensor("bq", [128, 1], F32, kind="ExternalInput")
    bk_in = nc.dram_tensor("bk", [128, 1], F32, kind="ExternalInput")
    out_t = nc.dram_tensor("out", [128, T], F32, kind="ExternalOutput")

    with tile.TileContext(nc) as tc:
        with (
            tc.tile_pool(name="const", bufs=1) as cpool,
            tc.tile_pool(name="work", bufs=2) as wpool,
            tc.tile_pool(name="psum", bufs=2, space="PSUM") as pspool,
        ):
            x_sb = cpool.tile([128, T], BF16)
            nc.sync.dma_start(x_sb[:], x_in[:])
            wq_sb = cpool.tile([128, 128], BF16)
            nc.sync.dma_start(wq_sb[:], wq_in[:])
            wk_sb = cpool.tile([128, 128], BF16)
            nc.sync.dma_start(wk_sb[:], wk_in[:])
            wv_sb = cpool.tile([128, 32], BF16)
            nc.sync.dma_start(wv_sb[:], wv_in[:])
            wp_sb = cpool.tile([32, 128], BF16)
            nc.sync.dma_start(wp_sb[:], wp_in[:])
            bq_sb = cpool.tile([128, 1], F32)
            nc.sync.dma_start(bq_sb[:], bq_in[:])
            bk_sb = cpool.tile([128, 1], F32)
            nc.sync.dma_start(bk_sb[:], bk_in[:])

            q_sb = cpool.tile([128, T], BF16)
            k_sb = cpool.tile([128, T], BF16)
            vT_sb = cpool.tile([128, 64 * (T // 128)], BF16)  # (128, 2048)
            an_sb = cpool.tile([32, T], BF16)

            nc.gpsimd.memset(vT_sb[:], 1.0)

            # ---- k/q projections, chunked so attention can start early ----
            # chunk order: k0,q0 first (needed by super 0), then the rest.
            def proj_chunks(chunks):
                ps = pspool.tile([128, 2048], F32, tag="ps")
                used = 0
                for wsb, bsb, dst, c in chunks:
                    m = used % 4
                    nc.tensor.matmul(
                        ps[:, m * 512:(m + 1) * 512], wsb[:],
                        x_sb[:, c * 512:(c + 1) * 512],
                        start=True, stop=True)
                    nc.vector.tensor_scalar_add(
                        dst[:, c * 512:(c + 1) * 512],
                        ps[:, m * 512:(m + 1) * 512], bsb[:])
                    used += 1
                    if used % 4 == 0:
                        ps = pspool.tile([128, 2048], F32, tag="ps")

            kc = [(wk_sb, bk_sb, k_sb, c) for c in range(8)]
            qc = [(wq_sb, bq_sb, q_sb, c) for c in range(8)]
            proj_chunks([kc[0], qc[0]] + kc[1:] + qc[1:])

            # ---- v transposed (+ ones blocks pre-set by memset) ----
            for half in range(2):
                ps = pspool.tile([128, 2048], F32, tag="ps")
                for j16 in range(16):
                    j = half * 16 + j16
                    nc.tensor.matmul(
                        ps[:, j16 * 32:(j16 + 1) * 32],
                        x_sb[:, j * 128:(j + 1) * 128],
                        wv_sb[:],
                        start=True, stop=True)
                # layout per 64-block: cols 0-31 v, 32-63 ones (denominator)
                src = ps[:, 0:512].rearrange("p (j c) -> p j c", c=32)
                dstv = vT_sb[:].rearrange("p (j c) -> p j c", c=64)
                nc.vector.tensor_copy(
                    dstv[:, half * 16:(half + 1) * 16, 0:32], src)

            if debug:
                nc.sync.dma_start(dbg["q"][:], q_sb[:])
                nc.sync.dma_start(dbg["k"][:], k_sb[:])
                nc.sync.dma_start(dbg["vT"][:], vT_sb[:])

            # ---- attention, software-pipelined over 64 supers ----
            state = {}   # jg -> (st, p_sb, a_sb)
            a_tiles = {}

            def emit_score_exp(jg):
                tb, j = divmod(jg, NSUP)
                if j == 0:
                    a_tiles[tb] = wpool.tile([64, 512], F32, tag="acc",
                                             name=f"a_sb_{tb}")
                st = pspool.tile([128, 2048], F32, tag="ps", name=f"st_{jg}")
                tsl = slice(tb * 512, (tb + 1) * 512)
                # bank 0 last: its WAR (prev flush) resolves latest
                for g in (1, 2, 3, 0):
                    sblk = 4 * j + g
                    nc.tensor.matmul(
                        st[:, g * 512:(g + 1) * 512],
                        k_sb[32 * g:32 * (g + 1), 128 * sblk:128 * (sblk + 1)],
                        q_sb[32 * g:32 * (g + 1), tsl],
                        start=True, stop=True,
                        tile_position=(32 * g, 0))
                p_sb = wpool.tile([128, 2048], BF16, tag="p")
                nc.scalar.activation(p_sb[:], st[:], Exp, scale=SCALE2)
                if debug and jg == 0:
                    nc.sync.dma_start(dbg["p"][:], p_sb[:])
                state[jg] = (st, p_sb)

            def emit_pv_flush(jg):
                tb, j = divmod(jg, NSUP)
                st, p_sb = state.pop(jg)
                a_sb = a_tiles[tb]
                for g in range(4):
                    sblk = 4 * j + g
                    nc.tensor.matmul(
                        st[0:64, 0:512],
                        vT_sb[:, 64 * sblk:64 * (sblk + 1)],
                        p_sb[:, g * 512:(g + 1) * 512],
                        start=(g == 0), stop=(g == 3))
                if j == 0:
                    nc.vector.tensor_copy(a_sb[:], st[0:64, 0:512])
                else:
                    nc.vector.tensor_add(a_sb[:], a_sb[:], st[0:64, 0:512])
                if j == NSUP - 1:
                    if debug and tb == 0:
                        o_dbg = wpool.tile([64, 512], F32, tag="odbg")
                        nc.vector.tensor_copy(o_dbg[:], a_sb[:])
                        nc.sync.dma_start(dbg["acc"][:], o_dbg[:])
                    tsl = slice(tb * 512, (tb + 1) * 512)
                    # custom-DVE reciprocal needs base partition 0: copy the
                    # denominator rows (32..63) down to a partition-0 tile.
                    dcp = wpool.tile([32, 512], F32, tag="dcp")
                    nc.vector.tensor_copy(dcp[:], a_sb[32:64, :])
                    rc = wpool.tile([32, 512], F32, tag="rc")
                    sc = wpool.tile([32, 512], F32, tag="sc")
                    nc.vector.reciprocal_approx_accurate(rc[:], dcp[:], sc[:])
                    nc.vector.tensor_mul(an_sb[:, tsl], a_sb[0:32, :], rc[:])

            for jg in range(NSUP * NSUP):
                emit_score_exp(jg)
                if jg >= 1:
                    emit_pv_flush(jg - 1)
            emit_pv_flush(NSUP * NSUP - 1)
            if debug:
                nc.sync.dma_start(dbg["an"][:], an_sb[:])

            # ---- output projection ----
            for half in range(2):
                ps = pspool.tile([128, 2048], F32, tag="ps")
                for m in range(4):
                    col = half * 2048 + m * 512
                    nc.tensor.matmul(
                        ps[:, m * 512:(m + 1) * 512],
                        wp_sb[:],
                        an_sb[:, col:col + 512],
                        start=True, stop=True)
                o_sb = wpool.tile([128, 2048], F32, tag="o")
                nc.vector.tensor_copy(o_sb[:], ps[:])
                nc.sync.dma_start(
                    out_t[:, half * 2048:(half + 1) * 2048], o_sb[:])

    nc.compile()
    return nc


def _get_nc(debug=False):
    key = ("nc", debug)
    if key not in _cache:
        _cache[key] = _build_nc(debug)
    return _cache[key]


def _make_in_maps(x_, w_qkv, b_qkv, w_proj):
    bf16 = ml_dtypes.bfloat16
    in_maps = []
    for core in range(N_CORES):
        b, g = divmod(core, NH)
        wq = w_qkv[96 * g:96 * g + 32]
        wk = w_qkv[96 * g + 32:96 * g + 64]
        wv = w_qkv[96 * g + 64:96 * g + 96]
        in_maps.append({
            "x": x_[b].astype(bf16),
            "wqT": np.ascontiguousarray(np.tile(wq, (4, 1)).T).astype(bf16),
            "wkT": np.ascontiguousarray(np.tile(wk, (4, 1)).T).astype(bf16),
            "wvT": np.ascontiguousarray(wv.T).astype(bf16),
            "wpT": np.ascontiguousarray(
                w_proj[:, 32 * g:32 * (g + 1)].T).astype(bf16),
            "bq": np.ascontiguousarray(
                np.tile(b_qkv[96 * g:96 * g + 32], 4).reshape(128, 1)),
            "bk": np.ascontiguousarray(
                np.tile(b_qkv[96 * g + 32:96 * g + 64], 4).reshape(128, 1)),
        })
    return in_maps


def _run(x, w_qkv, b_qkv, w_proj, b_proj, trace=False):
    from concourse.bass_utils import run_bass_kernel_spmd

    nc = _get_nc()
    x_ = np.ascontiguousarray(np.asarray(x, np.float32).reshape(B, C, T))
    w_qkv = np.asarray(w_qkv, np.float32)
    b_qkv = np.asarray(b_qkv, np.float32)
    w_proj = np.asarray(w_proj, np.float32)
    b_proj = np.asarray(b_proj, np.float32)

    in_maps = _make_in_maps(x_, w_qkv, b_qkv, w_proj)
    res = run_bass_kernel_spmd(nc, in_maps, core_ids=list(range(N_CORES)),
                               trace=trace)
    out = np.empty((B, C, T), np.float32)
    for b in range(B):
        acc = x_[b] + b_proj[:, None]
        for g in range(NH):
            wp = w_proj[:, 32 * g:32 * (g + 1)]
            bv = b_qkv[96 * g + 64:96 * g + 96]
            acc = acc + res.results[NH * b + g]["out"] + (wp @ bv)[:, None]
        out[b] = acc
    return out.reshape(B, C, Hh, Ww), res


def kernel(x, w_qkv, b_qkv, w_proj, b_proj):
    out, _ = _run(x, w_qkv, b_qkv, w_proj, b_proj, trace=False)
    return out.astype(np.asarray(x).dtype)


# revision 16
# speedup vs baseline: 1.2290x; 1.2290x over previous
"""Trainium2 Bass kernel for nn_Attention2d (sparse_attention).

Math (per reference):
  x: (2, 128, 64, 64); T = 4096 tokens; 4 heads x 32 channels.
  qkv 1x1-conv -> per-head attention over T -> 1x1-conv out proj -> residual.

Sharding: one (batch, head) pair per core (8 cores). Each core computes its
head's attention fully on-chip (flash-style streaming; no max-subtraction --
scores are O(5) so exp in fp32 is exact-safe) and returns the per-head
partial of the output projection; the host sums the 4 head partials per
batch and adds the residual + biases (exact).

Per-core structure (T=4096, t-blocks of 512, s-supers of 4x128):
  - q,k replicated x4 across partition groups so the K=32 score matmuls
    row-pack 4-wide into the PE array (tile_position) -> (128s, 512t)
    score block per bank, 4 banks per super, single-buffered ("st").
  - ScalarE exp's the 4 banks (2048 cols) in one instruction -> bf16 P in
    SBUF. ScalarE is the theoretical pacer: 16.7M exps/core at
    1 elem/cycle/lane.
  - v is produced directly transposed (x_block stationary) with a 32-wide
    ones block appended, so each PV matmul also emits the softmax
    denominator replicated over partitions 32..63.
  - PV accumulates over the whole t-block (32 matmuls) into a dedicated
    1-bank PSUM accumulator (2 rotating banks across t-blocks) -> no
    per-super VectorE flush on the critical path.
  - PSUM budget: st 4 banks + pv 2 + pp (prologue/epilogue ping-pong) 2.
  - Emission is software-pipelined: S_T(j+1) before PV(j), so the Tensor
    FIFO stalls only on exp(j), and exp(j+1) starts after S_T(j+1)'s 604ns.
"""

import numpy as np
import ml_dtypes

B, C, Hh, Ww = 2, 128, 64, 64
T = Hh * Ww          # 4096
NH, CH = 4, 32
SCALE2 = float(1.0 / np.sqrt(CH))
N_CORES = 8
NSUP = T // 512      # 8 supers per t-block, 8 t-blocks

_cache = {}


def _build_nc(debug=False):
    import concourse.tile as tile
    from concourse import bacc, mybir

    BF16 = mybir.dt.bfloat16
    F32 = mybir.dt.float32
    Exp = mybir.ActivationFunctionType.Exp

    nc = bacc.Bacc("TRN2", target_bir_lowering=False, debug=False,
                   num_devices=N_CORES)
    dbg = {}
    if debug:
        dbg["q"] = nc.dram_tensor("dq", [128, T], BF16, kind="ExternalOutput")
        dbg["k"] = nc.dram_tensor("dk", [128, T], BF16, kind="ExternalOutput")
        dbg["vT"] = nc.dram_tensor("dvT", [128, 2048], BF16, kind="ExternalOutput")
        dbg["an"] = nc.dram_tensor("dan", [32, T], BF16, kind="ExternalOutput")
        dbg["acc"] = nc.dram_tensor("dacc", [64, 512], F32, kind="ExternalOutput")
        dbg["p"] = nc.dram_tensor("dp", [128, 2048], BF16, kind="ExternalOutput")

    x_in = nc.dram_tensor("x", [128, T], BF16, kind="ExternalInput")
    wq_in = nc.dram_tensor("wqT", [128, 128], BF16, kind="ExternalInput")
    wk_in = nc.dram_tensor("wkT", [128, 128], BF16, kind="ExternalInput")
    wv_in = nc.dram_tensor("wvT", [128, 32], BF16, kind="ExternalInput")
    wp_in = nc.dram_tensor("wpT", [32, 128], BF16, kind="ExternalInput")
    bq_in = nc.dram_tensor("bq", [128, 1], F32, kind="ExternalInput")
    bk_in = nc.dram_tensor("bk", [128, 1], F32, kind="ExternalInput")
    out_t = nc.dram_tensor("out", [128, T], F32, kind="ExternalOutput")

    with tile.TileContext(nc) as tc:
        with (
            tc.tile_pool(name="const", bufs=1) as cpool,
            tc.tile_pool(name="work", bufs=2) as wpool,
            tc.tile_pool(name="psum", bufs=1, space="PSUM") as pspool,
        ):
            x_sb = cpool.tile([128, T], BF16)
            nc.sync.dma_start(x_sb[:], x_in[:])
            wq_sb = cpool.tile([128, 128], BF16)
            nc.sync.dma_start(wq_sb[:], wq_in[:])
            wk_sb = cpool.tile([128, 128], BF16)
            nc.sync.dma_start(wk_sb[:], wk_in[:])
            wv_sb = cpool.tile([128, 32], BF16)
            nc.sync.dma_start(wv_sb[:], wv_in[:])
            wp_sb = cpool.tile([32, 128], BF16)
            nc.sync.dma_start(wp_sb[:], wp_in[:])
            bq_sb = cpool.tile([128, 1], F32)
            nc.sync.dma_start(bq_sb[:], bq_in[:])
            bk_sb = cpool.tile([128, 1], F32)
            nc.sync.dma_start(bk_sb[:], bk_in[:])

            q_sb = cpool.tile([128, T], BF16)
            k_sb = cpool.tile([128, T], BF16)
            vT_sb = cpool.tile([128, 64 * (T // 128)], BF16)  # (128, 2048)
            an_sb = cpool.tile([32, T], BF16)

            nc.gpsimd.memset(vT_sb[:], 1.0)

            def pp_tile(nm):
                return pspool.tile([128, 512], F32, tag="pp", bufs=2, name=nm)

            # ---- k/q projections through the 1-bank ping-pong pool ----
            for wsb, bsb, dst, pref in ((wk_sb, bk_sb, k_sb, "k"),
                                        (wq_sb, bq_sb, q_sb, "q")):
                for c in range(8):
                    ps = pp_tile(f"pp_{pref}{c}")
                    nc.tensor.matmul(ps[:], wsb[:],
                                     x_sb[:, c * 512:(c + 1) * 512],
                                     start=True, stop=True)
                    nc.vector.tensor_scalar_add(
                        dst[:, c * 512:(c + 1) * 512], ps[:], bsb[:])

            # ---- v transposed (cols 0-31 of each 64-block; 32-63 ones) ----
            for half in range(2):
                ps = pp_tile(f"pp_v{half}")
                for j16 in range(16):
                    j = half * 16 + j16
                    nc.tensor.matmul(
                        ps[:, j16 * 32:(j16 + 1) * 32],
                        x_sb[:, j * 128:(j + 1) * 128],
                        wv_sb[:],
                        start=True, stop=True)
                src = ps[:].rearrange("p (j c) -> p j c", c=32)
                dstv = vT_sb[:].rearrange("p (j c) -> p j c", c=64)
                nc.vector.tensor_copy(
                    dstv[:, half * 16:(half + 1) * 16, 0:32], src)

            if debug:
                nc.sync.dma_start(dbg["q"][:], q_sb[:])
                nc.sync.dma_start(dbg["k"][:], k_sb[:])
                nc.sync.dma_start(dbg["vT"][:], vT_sb[:])

            # ---- attention, software-pipelined over 64 supers ----
            state = {}
            pv_tiles = {}

            def emit_score_exp(jg):
                tb, j = divmod(jg, NSUP)
                if j == 0:
                    pv_tiles[tb] = pspool.tile(
                        [64, 512], F32, tag="pv", bufs=2, name=f"pv_{tb}")
                st = pspool.tile([128, 2048], F32, tag="st", bufs=1,
                                 name=f"st_{jg}")
                tsl = slice(tb * 512, (tb + 1) * 512)
                for g in range(4):
                    sblk = 4 * j + g
                    nc.tensor.matmul(
                        st[:, g * 512:(g + 1) * 512],
                        k_sb[32 * g:32 * (g + 1), 128 * sblk:128 * (sblk + 1)],
                        q_sb[32 * g:32 * (g + 1), tsl],
                        start=True, stop=True,
                        tile_position=(32 * g, 0))
                p_sb = wpool.tile([128, 2048], BF16, tag="p")
                nc.scalar.activation(p_sb[:], st[:], Exp, scale=SCALE2)
                if debug and jg == 0:
                    nc.sync.dma_start(dbg["p"][:], p_sb[:])
                state[jg] = p_sb

            def emit_pv(jg):
                tb, j = divmod(jg, NSUP)
                p_sb = state.pop(jg)
                pv = pv_tiles[tb]
                for g in range(4):
                    sblk = 4 * j + g
                    nc.tensor.matmul(
                        pv[:], vT_sb[:, 64 * sblk:64 * (sblk + 1)],
                        p_sb[:, g * 512:(g + 1) * 512],
                        start=(j == 0 and g == 0), stop=(j == NSUP - 1 and g == 3),
                        skip_group_check=True)
                if j == NSUP - 1:
                    # t-block epilogue, all off the critical path (VectorE)
                    a_sb = wpool.tile([64, 512], F32, tag="acc")
                    nc.vector.tensor_copy(a_sb[:], pv[:])
                    if debug and tb == 0:
                        nc.sync.dma_start(dbg["acc"][:], a_sb[:])
                    tsl = slice(tb * 512, (tb + 1) * 512)
                    dcp = wpool.tile([32, 512], F32, tag="dcp")
                    nc.vector.tensor_copy(dcp[:], a_sb[32:64, :])
                    rc = wpool.tile([32, 512], F32, tag="rc")
                    sc = wpool.tile([32, 512], F32, tag="sc")
                    nc.vector.reciprocal_approx_accurate(rc[:], dcp[:], sc[:])
                    nc.vector.tensor_mul(an_sb[:, tsl], a_sb[0:32, :], rc[:])

            for jg in range(NSUP * NSUP):
                emit_score_exp(jg)
                if jg >= 1:
                    emit_pv(jg - 1)
            emit_pv(NSUP * NSUP - 1)
            if debug:
                nc.sync.dma_start(dbg["an"][:], an_sb[:])

            # ---- output projection, chunked through pp with DMA overlap ----
            for c in range(8):
                ps = pp_tile(f"pp_o{c}")
                nc.tensor.matmul(ps[:], wp_sb[:],
                                 an_sb[:, c * 512:(c + 1) * 512],
                                 start=True, stop=True)
                o_sb = wpool.tile([128, 512], F32, tag="o")
                nc.vector.tensor_copy(o_sb[:], ps[:])
                nc.sync.dma_start(out_t[:, c * 512:(c + 1) * 512], o_sb[:])

    nc.compile()
    return nc


def _get_nc(debug=False):
    key = ("nc", debug)
    if key not in _cache:
        _cache[key] = _build_nc(debug)
    return _cache[key]


def _make_in_maps(x_, w_qkv, b_qkv, w_proj):
    bf16 = ml_dtypes.bfloat16
    in_maps = []
    for core in range(N_CORES):
        b, g = divmod(core, NH)
        wq = w_qkv[96 * g:96 * g + 32]
        wk = w_qkv[96 * g + 32:96 * g + 64]
        wv = w_qkv[96 * g + 64:96 * g + 96]
        in_maps.append({
            "x": x_[b].astype(bf16),
            "wqT": np.ascontiguousarray(np.tile(wq, (4, 1)).T).astype(bf16),
            "wkT": np.ascontiguousarray(np.tile(wk, (4, 1)).T).astype(bf16),
            "wvT": np.ascontiguousarray(wv.T).astype(bf16),
            "wpT": np.ascontiguousarray(
                w_proj[:, 32 * g:32 * (g + 1)].T).astype(bf16),
            "bq": np.ascontiguousarray(
                np.tile(b_qkv[96 * g:96 * g + 32], 4).reshape(128, 1)),
            "bk": np.ascontiguousarray(
                np.tile(b_qkv[96 * g + 32:96 * g + 64], 4).reshape(128, 1)),
        })
    return in_maps


def _run(x, w_qkv, b_qkv, w_proj, b_proj, trace=False):
    from concourse.bass_utils import run_bass_kernel_spmd

    nc = _get_nc()
    x_ = np.ascontiguousarray(np.asarray(x, np.float32).reshape(B, C, T))
    w_qkv = np.asarray(w_qkv, np.float32)
    b_qkv = np.asarray(b_qkv, np.float32)
    w_proj = np.asarray(w_proj, np.float32)
    b_proj = np.asarray(b_proj, np.float32)

    in_maps = _make_in_maps(x_, w_qkv, b_qkv, w_proj)
    res = run_bass_kernel_spmd(nc, in_maps, core_ids=list(range(N_CORES)),
                               trace=trace)
    out = np.empty((B, C, T), np.float32)
    for b in range(B):
        acc = x_[b] + b_proj[:, None]
        for g in range(NH):
            wp = w_proj[:, 32 * g:32 * (g + 1)]
            bv = b_qkv[96 * g + 64:96 * g + 96]
            acc = acc + res.results[NH * b + g]["out"] + (wp @ bv)[:, None]
        out[b] = acc
    return out.reshape(B, C, Hh, Ww), res


def kernel(x, w_qkv, b_qkv, w_proj, b_proj):
    out, _ = _run(x, w_qkv, b_qkv, w_proj, b_proj, trace=False)
    return out.astype(np.asarray(x).dtype)
